# revision 41
# baseline (speedup 1.0000x reference)
"""HMM window log-likelihood on 8 NeuronCores (data-parallel over batch).

Math: reference computes, per batch column b,
    y[b] = exp(logsumexp_i x_T[b,i]),  x via log-space forward recursion.
Equivalently in linear space with row-normalized transition matrices
Wn_t = exp(w[t-1]) / rowsum, emission table L = softmax(distros, axis=1):
    y[b] = 1^T diag(em_{L-1}) Wn_{L-1} ... diag(em_1) Wn_1 em_0
We split the chain in the middle (meet at m=127) and run a FORWARD
recursion a_t = em_t . (Wn_t a_{t-1}) from t=0 and a BACKWARD recursion
beta_{t-1} = Wn_t^T (em_t . beta_t) from t=L-1 simultaneously, halving
the serial dependency depth:   y[b] = sum_i a_m[i,b] beta_m[i,b].
Per-step rescale factors (host-computed from column 0, f64) and the row
normalization are folded into per-step emissions, which are precomputed
on the HOST in bf16 and DMA-streamed (emf fwd / emb bwd) - the PE then
only runs the transition matmuls. TRN2 engine rules: DVE may read ONE
PSUM operand per op, GPSIMD/Pool may read none, Activation scales are
per-partition only. So each per-step multiply c = mm . em runs as a
single fused DVE op (mm from PSUM, host-streamed em from SBUF); with
BH0=BC both directions' full-width multiplies go through DVE, which the
scheduler overlaps best (an optional Act-copy + Pool-multiply strip for
columns [BH0,BC) exists but benchmarked slower).
Device returns colsum[b] = y[b] * prod(g); host: lnY = log(colsum)+C.
The true lnY is ~ -584.6 for these inputs, so y underflows f32 to 0.0 -
exactly matching the reference (which also underflows in f32).
"""
import sys, os
for p in ("/opt/trn_rl_repo",):
    if p not in sys.path:
        sys.path.insert(0, p)
import numpy as np
import ml_dtypes

from concourse import bass, bacc, mybir
from concourse.tile import TileContext
from concourse.bass_utils import run_bass_kernel_spmd

W, L, B, NB = 128, 256, 4096, 10
NCORES = 8
BC = B // NCORES          # 512 batch cols per core
BH0 = 512                 # DVE strip width (Act->Pool strip gets BC - BH0)
M = 127                   # meet step: fwd covers t<=M, bwd t>M
TBLK = 4                  # em streaming block (t's per DMA, per direction)

LAST_LNY = None           # debug: device-derived lnY per batch col
LAST_RESULTS = None       # debug: raw BassKernelResults

_CACHED = None            # (nc,) build cache


def _build_nc():
    nc = bacc.Bacc("TRN2", target_bir_lowering=False, debug=False,
                   num_devices=NCORES)
    bf16, f32 = mybir.dt.bfloat16, mybir.dt.float32

    wt = nc.dram_tensor("wt", [W, L - 1, W], bf16, kind="ExternalInput")
    emf = nc.dram_tensor("emf", [W, L // 2, BC], bf16, kind="ExternalInput")
    emb = nc.dram_tensor("emb", [W, L // 2, BC], bf16, kind="ExternalInput")
    ones = nc.dram_tensor("ones", [W, 1], bf16, kind="ExternalInput")
    colsum = nc.dram_tensor("colsum", [1, BC], f32, kind="ExternalOutput")

    Copy = mybir.ActivationFunctionType.Copy
    Mult = mybir.AluOpType.mult

    N0, N1 = BH0, BC - BH0
    NKB = L // (2 * TBLK)     # em blocks per direction
    KMAX = L // 2

    with TileContext(nc) as tc:
        with tc.sbuf_pool(name="sb", bufs=2) as sb, \
                tc.psum_pool(name="ps", bufs=2) as ps:
            ones_sb = sb.tile([W, 1], bf16, bufs=1)
            nc.sync.dma_start(ones_sb, ones.ap())

            # streamed emissions: emf[:,k,:] = em_k, emb[:,kk,:] = em_{L-1-kk}
            # block layout: two small starter blocks (2 and 6 steps) so the
            # first iterations start ~4us earlier, then regular TBLK blocks.
            BLKS = [(0, 1), (1, 2), (3, 5)] + [
                (s, 4) for s in range(8, L // 2, 4)]
            NKB = len(BLKS)
            BLK_OF = []
            for bi, (s0, cnt) in enumerate(BLKS):
                BLK_OF += [(bi, s0)] * cnt
            eF = [None] * NKB
            eB = [None] * NKB

            def load_blk(blk, eng_b=None):
                s0, cnt = BLKS[blk]
                issue_wt(cnt + 4)
                ef_t = sb.tile([W, cnt, BC], bf16, tag=f"eF{cnt}",
                               bufs=4 if cnt == 4 else 1)
                nc.sync.dma_start(ef_t, emf.ap()[:, s0:s0 + cnt, :])
                eF[blk] = ef_t
                eb_t = sb.tile([W, cnt, BC], bf16, tag=f"eB{cnt}",
                               bufs=4 if cnt == 4 else 1)
                (eng_b or nc.scalar).dma_start(eb_t, emb.ap()[:, s0:s0 + cnt, :])
                eB[blk] = eb_t

            # all 255 transition matrices resident, streamed in small chunks
            # interleaved with the bwd-emission blocks on the Activation
            # engine's HWDGE queue, staying just ahead of consumption (the
            # SP queue carries only the fwd-emission stream)
            wt_sb = sb.tile([W, L - 1, W], bf16, bufs=1)
            wt_cur = [0, L - 2]   # fwd ascending from tau=0, bwd descending

            def issue_wt(n):
                lo0, hi1 = wt_cur
                lo_cnt = min(n, M - lo0)
                hi_cnt = min(n, hi1 - M + 1)
                if hi_cnt > 0:
                    nc.scalar.dma_start(
                        wt_sb[:, hi1 - hi_cnt + 1:hi1 + 1, :],
                        wt.ap()[:, hi1 - hi_cnt + 1:hi1 + 1, :])
                if lo_cnt > 0:
                    nc.scalar.dma_start(wt_sb[:, lo0:lo0 + lo_cnt, :],
                                        wt.ap()[:, lo0:lo0 + lo_cnt, :])
                wt_cur[0] = lo0 + lo_cnt
                wt_cur[1] = hi1 - hi_cnt

            load_blk(0)
            load_blk(1)
            load_blk(2)

            # warm the PE p-state with dummy matmuls (ones x first wt slice)
            # so the first real transition matmuls run at full clock

            aF = None             # fwd state a_t   (SBUF bf16, [W, BC])
            cB = None             # bwd carry c_t   (SBUF bf16, [W, BC])
            mmF = ps.tile([W, BC], f32, tag="mmF", bufs=1)
            bB = ps.tile([W, BC], f32, tag="bB", bufs=1)

            for k in range(KMAX):
                blk, s0b = BLK_OF[k]
                ti = k - s0b
                # single-block lookahead: the conservative per-queue DMA
                # fence then spans at most one in-flight block (~3us)
                if ti == 0 and blk >= 2 and blk + 1 < NKB \
                        and eF[blk + 1] is None:
                    load_blk(blk + 1)
                tB = L - 1 - k
                efk, ebk = eF[blk], eB[blk]
                if k == 0:
                    aF = efk[:, 0, :]
                    cB = ebk[:, 0, :]
                    continue
                # one full-width transition matmul per direction
                nc.tensor.matmul(mmF, wt_sb[:, k - 1, :], aF,
                                 start=True, stop=True)
                nc.tensor.matmul(bB, wt_sb[:, tB, :], cB,
                                 start=True, stop=True)
                # consumers split by column range: DVE fuses evacuate+
                # multiply for [0:N0] (one PSUM operand, 1x rate); for
                # [N0:BC] the Act engine evacuates to SBUF bf16 and DVE
                # multiplies all-SBUF 2-byte operands at its fast rate
                a_new = sb.tile([W, BC], bf16, tag="aF0", bufs=2)
                c_new = sb.tile([W, BC], bf16, tag="cB0", bufs=2)
                nc.vector.tensor_mul(a_new[:, 0:N0], mmF[:, 0:N0],
                                     efk[:, ti, 0:N0])
                nc.vector.tensor_mul(c_new[:, 0:N0], bB[:, 0:N0],
                                     ebk[:, ti, 0:N0])
                if N1:
                    cpF = sb.tile([W, N1], bf16, tag="cpF", bufs=2)
                    nc.scalar.activation(cpF, mmF[:, N0:BC], Copy)
                    cpB = sb.tile([W, N1], bf16, tag="cpB", bufs=2)
                    nc.scalar.activation(cpB, bB[:, N0:BC], Copy)
                    nc.vector.tensor_mul(a_new[:, N0:BC], cpF,
                                         efk[:, ti, N0:BC])
                    nc.vector.tensor_mul(c_new[:, N0:BC], cpB,
                                         ebk[:, ti, N0:BC])
                aF, cB = a_new, c_new

            # meet: beta_M = Wn_{M+1}^T c_{M+1};  colsum = 1^T (a_M . beta_M)
            cs_ps = ps.tile([1, BC], f32, tag="cs", bufs=1)
            nc.tensor.matmul(bB, wt_sb[:, M, :], cB,
                             start=True, stop=True)
            q_sb = sb.tile([W, BC], bf16, tag="q0", bufs=1)
            nc.vector.tensor_mul(q_sb, bB, aF)
            nc.tensor.matmul(cs_ps, ones_sb, q_sb,
                             start=True, stop=True)

            cs_sb = sb.tile([1, BC], f32, bufs=1)
            nc.vector.tensor_copy(cs_sb, cs_ps)
            nc.sync.dma_start(colsum.ap(), cs_sb)
    nc.compile()
    return nc


def _host_prep(data, input_distros, dense_layer_weights):
    f64 = np.float64
    we = np.exp(dense_layer_weights.astype(f64))           # (255,W,W)
    rowsum = we.sum(axis=2)                                # (255,W)
    recip = 1.0 / rowsum
    d = input_distros.astype(f64)
    d = d - d.max(axis=1, keepdims=True)
    e = np.exp(d)
    Ll = e / e.sum(axis=1, keepdims=True)                  # (W,NB) softmax rows
    # bins exactly as reference: floor(v / 0.1) in f32
    bins = np.minimum(NB - 1, np.floor(
        data / np.float32(0.1)).astype(np.int32))          # (B,L)

    rsg = np.ones((W, L), dtype=f64)
    Cacc = 0.0

    # forward col-0 f64 pass -> per-step rescale gF_k for t=1..M
    a = Ll[np.arange(W), bins[0, 0]]
    for k in range(1, M + 1):
        raw = Ll[np.arange(W), bins[0, k]] * recip[k - 1] * (we[k - 1] @ a)
        f = raw.max()
        Cacc += np.log(f)
        a = raw / f
        rsg[:, k] = recip[k - 1] / f

    # backward col-0 f64 pass -> per-step rescale gB_t for t=L-1..M+1
    beta = np.ones(W, dtype=f64)
    for t in range(L - 1, M, -1):
        c = Ll[np.arange(W), bins[0, t]] * beta * recip[t - 1]
        tmp = we[t - 1].T @ c
        f = tmp.max()
        Cacc += np.log(f)
        beta = tmp / f
        rsg[:, t] = recip[t - 1] / f

    # host-precomputed per-step emissions (rescale folded), bf16:
    #   emf[i, k, b] = L[i, bins[b, k]] * rsg[i, k]           (t = k)
    #   emb[i, kk, b] = L[i, bins[b, L-1-kk]] * rsg[i, L-1-kk]
    iw = np.arange(W)[:, None, None]
    tf = np.arange(L // 2)
    emf = (Ll[iw, bins.T[None, tf, :]] * rsg[:, tf][:, :, None]
           ).astype(ml_dtypes.bfloat16)                    # (W, L/2, B)
    tb = np.arange(L - 1, L // 2 - 1, -1)
    emb = (Ll[iw, bins.T[None, tb, :]] * rsg[:, tb][:, :, None]
           ).astype(ml_dtypes.bfloat16)                    # (W, L/2, B)

    # mixed-layout transitions: tau < M fwd (lhsT = we[tau]^T),
    # tau >= M bwd (lhsT = we[tau])
    wt2 = np.empty((W, L - 1, W), dtype=f64)
    wt2[:, :M, :] = we[:M].transpose(2, 0, 1)
    wt2[:, M:, :] = we[M:].transpose(1, 0, 2)
    wt = np.ascontiguousarray(wt2).astype(ml_dtypes.bfloat16)

    ones_v = np.ones((W, 1), dtype=ml_dtypes.bfloat16)
    return wt, emf, emb, ones_v, Cacc


def kernel(data, input_distros, dense_layer_weights):
    global LAST_LNY, LAST_RESULTS, _CACHED
    wt, emf, emb, ones_v, Cacc = _host_prep(
        np.asarray(data), np.asarray(input_distros),
        np.asarray(dense_layer_weights))

    if _CACHED is None:
        _CACHED = _build_nc()
    nc = _CACHED

    in_maps = []
    for c in range(NCORES):
        sl = slice(c * BC, (c + 1) * BC)
        in_maps.append({
            "wt": wt, "ones": ones_v,
            "emf": np.ascontiguousarray(emf[:, :, sl]),
            "emb": np.ascontiguousarray(emb[:, :, sl]),
        })
    res = run_bass_kernel_spmd(
        nc, in_maps, core_ids=list(range(NCORES)),
        trace=bool(int(os.environ.get("KERNEL_TRACE", "0"))))
    LAST_RESULTS = res
    cs = np.concatenate([res.results[c]["colsum"].reshape(-1)
                         for c in range(NCORES)])           # (B,)
    lnY = np.log(cs.astype(np.float64)) + Cacc
    LAST_LNY = lnY
    y = np.exp(lnY).astype(np.float32).reshape(B, 1)
    return y
